# revision 30
# baseline (speedup 1.0000x reference)
"""GQA attention (B=2,S=2048,E=2048,H=16,KV=4,D=128, RoPE, causal) on 8 trn2 cores.

Sharding: core c = (b = c//4, kv = c%4). Tensor-parallel over kv-head groups
(Wq cols / Wk,Wv cols / Wo rows) x data-parallel over batch. Each core computes
a full [S, E] partial output (its head group's contribution); host sums the 4
partials per batch element.

All device inputs are bf16, host-packed to the on-chip layout ([128, ...]
with fully-contiguous partition rows) so every DMA is a fast same-dtype
hardware-DGE transfer; x on the sync queue, weights on the scalar queue.
The next block's k-projection chain is emitted between attention and the
output projection so the PE stays busy across phase boundaries. Every
matmul has its contraction dim on partitions:
  qT/kT [d, s] = Wq_chunk.T @ xT    (PSUM accum over e-chunks)
  RoPE rotation on-chip: rot = perm.T @ qraw (one [128,128] perm matmul)
  v     [s, d] = xT_chunk.T @ Wv
  scoresT [sk, sq] = kT_chunk.T @ qT_block  (suffix-width in diagonal strip)
  softmax without max-subtraction; rowsum+broadcast per key chunk in one
  PE matmul (ones[128,128] stationary, PSUM-accumulated over chunks);
  1/rowsum via reciprocal_approx_fast on [128,512]
  outT [d, sq] += v_chunk.T @ expT
  y [sq, e] += outT_norm_chunk.T @ Wo_head  (accum over 4 heads)

All matmuls bf16 (fp32 PSUM accum); y written bf16, host sums in fp32.
"""
import sys
sys.path.insert(0, "/opt/trn_rl_repo")
import numpy as np
import ml_dtypes

BF16 = ml_dtypes.bfloat16

B, S, E = 2, 2048, 2048
H, KV, D = 16, 4, 128
G = H // KV          # 4 q heads per kv head / core
THETA = 10000.0
P = 128
NE = E // P          # 16 e-chunks
NB = 4               # s-blocks per core loop
BS = S // NB         # 512
NSC = S // P         # 16 s-chunks

_CACHE = {}


def _build():
    if "nc" in _CACHE:
        return _CACHE["nc"]
    import concourse.bass as bass
    import concourse.tile as tile
    from concourse import mybir, bacc

    f32 = mybir.dt.float32
    bf16 = mybir.dt.bfloat16
    EXP = mybir.ActivationFunctionType.Exp
    SCALE = 1.0 / np.sqrt(D)

    nc = bacc.Bacc("TRN2", target_bir_lowering=False, debug=False)
    # all packed host-side to the on-chip layout: fully-contiguous rows
    xT_d = nc.declare_dram_parameter("xP", [P, NB * NE * BS], bf16,
                                     isOutput=False)
    wq_d = nc.declare_dram_parameter("wqP", [P, NE * G * D], bf16,
                                     isOutput=False)
    wk_d = nc.declare_dram_parameter("wkP", [P, NE * D], bf16, isOutput=False)
    wv_d = nc.declare_dram_parameter("wvP", [P, NE * D], bf16, isOutput=False)
    wo_d = nc.declare_dram_parameter("woP", [P, G * E], bf16, isOutput=False)
    cs_d = nc.declare_dram_parameter("csT", [P, S], bf16, isOutput=False)
    sn_d = nc.declare_dram_parameter("snT", [P, S], bf16, isOutput=False)
    pm_d = nc.declare_dram_parameter("pm", [P, P], bf16, isOutput=False)
    tri_d = nc.declare_dram_parameter("tri", [P, P], bf16, isOutput=False)
    y_d = nc.declare_dram_parameter("y", [S, E], bf16, isOutput=True)

    with tile.TileContext(nc) as tc, \
         nc.allow_low_precision(reason="bf16 matmul pipeline"):
        import contextlib
        with contextlib.ExitStack() as ctx:
            cst = ctx.enter_context(tc.tile_pool(name="cst", bufs=1))
            xp = ctx.enter_context(tc.tile_pool(name="xp", bufs=4))
            wp = ctx.enter_context(tc.tile_pool(name="wp", bufs=1))
            kvp = ctx.enter_context(tc.tile_pool(name="kvp", bufs=1))
            vp = ctx.enter_context(tc.tile_pool(name="vp", bufs=16))
            qtp = ctx.enter_context(tc.tile_pool(name="qtp", bufs=5))
            rawp = ctx.enter_context(tc.tile_pool(name="rawp", bufs=3))
            rpp = ctx.enter_context(tc.tile_pool(name="rpp", bufs=4))
            exp_p = ctx.enter_context(tc.tile_pool(name="exp", bufs=4))
            rcp = ctx.enter_context(tc.tile_pool(name="rcp", bufs=2))
            otp = ctx.enter_context(tc.tile_pool(name="otp", bufs=6))
            yp = ctx.enter_context(tc.tile_pool(name="yp", bufs=2))
            psA = ctx.enter_context(tc.tile_pool(name="psA", bufs=3, space="PSUM"))
            psO = ctx.enter_context(tc.tile_pool(name="psO", bufs=2, space="PSUM"))
            psY = ctx.enter_context(tc.tile_pool(name="psY", bufs=3, space="PSUM"))

            # ---- resident tiles ----
            xtj = [xp.tile([P, NE * BS], bf16, tag="xt", name=f"xt{j}")
                   for j in range(NB)]
            wq_all = wp.tile([P, NE * G * D], bf16, tag="wq")
            wk_all = wp.tile([P, NE * D], bf16, tag="wk")
            wv_all = wp.tile([P, NE * D], bf16, tag="wv")
            wo_all = wp.tile([P, G * E], bf16, tag="wo")
            cs_sb = cst.tile([P, S], bf16, tag="cs")
            sn_sb = cst.tile([P, S], bf16, tag="sn")
            pm_sb = cst.tile([P, P], bf16, tag="pm")
            tri_sb = cst.tile([P, P], bf16, tag="tri")
            ones_sb = cst.tile([P, P], bf16, tag="ones")
            nc.vector.memset(ones_sb[:], 1.0)
            kT_sb = kvp.tile([P, S], bf16, tag="kT")
            v_sb = [vp.tile([P, D], bf16, tag="v", name=f"v{i}")
                    for i in range(NSC)]

            def xs(j, e, lo=0, hi=BS):
                """moving slice of block-j x panel, e-chunk rows."""
                return xtj[j][:, e * BS + lo:e * BS + hi]

            # ---- DMA: everything is a fully-contiguous 2D transfer
            # (arrays are host-packed to the on-chip layout). x on the
            # sync queue, weights on the scalar queue. ----
            def load_x(j, e0, e1):
                lo, hi = j * NE * BS + e0 * BS, j * NE * BS + e1 * BS
                nc.sync.dma_start(xtj[j][:, e0 * BS:e1 * BS], xT_d[:, lo:hi])

            GD = G * D
            load_x(0, 0, 4)
            nc.sync.dma_start(wk_all[:, 0:8 * D], wk_d[:, 0:8 * D])
            nc.sync.dma_start(wk_all[:, 8 * D:], wk_d[:, 8 * D:])
            nc.scalar.dma_start(pm_sb[:], pm_d[:])
            nc.scalar.dma_start(tri_sb[:], tri_d[:])
            nc.scalar.dma_start(wq_all[:, 0:4 * GD], wq_d[:, 0:4 * GD])
            load_x(0, 4, 8)
            load_x(0, 8, 12)
            nc.scalar.dma_start(xtj[0][:, 12 * BS:16 * BS],
                                xT_d[:, 12 * BS:16 * BS])
            nc.sync.dma_start(cs_sb[:], cs_d[:])
            nc.sync.dma_start(sn_sb[:], sn_d[:])
            for q in range(1, 4):
                nc.scalar.dma_start(wq_all[:, 4 * q * GD:4 * (q + 1) * GD],
                                    wq_d[:, 4 * q * GD:4 * (q + 1) * GD])
            for j in range(1, NB):
                load_x(j, 0, NE)
            nc.scalar.dma_start(wv_all[:], wv_d[:])
            nc.scalar.dma_start(wo_all[:], wo_d[:])

            def rope_evac(dst, ps, ps2, j):
                """dst (bf16) = ps*cos + ps2*sin at abs position j*BS."""
                cs = cs_sb[:, j * BS:(j + 1) * BS]
                sn = sn_sb[:, j * BS:(j + 1) * BS]
                t1 = rpp.tile([P, BS], f32, tag="rp")
                nc.vector.tensor_mul(t1[:], ps[:], cs)
                t2 = rpp.tile([P, BS], f32, tag="rp")
                nc.vector.tensor_mul(t2[:], ps2[:], sn)
                nc.vector.tensor_add(dst, t1[:], t2[:])

            def proj_k(j):
                js = slice(j * BS, (j + 1) * BS)
                ps_k = psA.tile([P, BS], f32, tag="a", name=f"psk{j}")
                for e in range(NE):
                    nc.tensor.matmul(ps_k[:], wk_all[:, e * D:(e + 1) * D],
                                     xs(j, e),
                                     start=(e == 0), stop=(e == NE - 1))
                kraw = rawp.tile([P, BS], bf16, tag="raw", name=f"kraw{j}")
                nc.scalar.copy(kraw[:], ps_k[:])
                return ps_k, kraw

            def proj_qv(j, kctx):
                """q chains with trailing perm+rope (starting with k's),
                then v chains."""
                js = slice(j * BS, (j + 1) * BS)
                ps_k, kraw = kctx
                qT = []
                prev = ("k", ps_k, kraw, None)
                for h in range(G):
                    ps_q = psA.tile([P, BS], f32, tag="a", name=f"psq{h}")
                    for e in range(NE):
                        nc.tensor.matmul(
                            ps_q[:],
                            wq_all[:, e * G * D + h * D:e * G * D + (h + 1) * D],
                            xs(j, e),
                            start=(e == 0), stop=(e == NE - 1))
                    qraw = rawp.tile([P, BS], bf16, tag="raw", name=f"qraw{h}")
                    nc.scalar.copy(qraw[:], ps_q[:])
                    kind, psp, raw, dst = prev
                    ps2 = psA.tile([P, BS], f32, tag="a", name=f"ps2{h}")
                    nc.tensor.matmul(ps2[:], pm_sb[:], raw[:],
                                     start=True, stop=True)
                    if kind == "k":
                        rope_evac(kT_sb[:, js], psp, ps2, j)
                    else:
                        rope_evac(dst[:], psp, ps2, j)
                    qh = qtp.tile([P, BS], bf16, tag="qT", name=f"qT{h}")
                    qT.append(qh)
                    prev = ("q", ps_q, qraw, qh)

                kind, psp, raw, dst = prev
                ps2 = psA.tile([P, BS], f32, tag="a", name="ps2l")
                nc.tensor.matmul(ps2[:], pm_sb[:], raw[:], start=True,
                                 stop=True)
                rope_evac(dst[:], psp, ps2, j)

                for sc in range(4):
                    scg = 4 * j + sc
                    ps_v = psA.tile([P, D], f32, tag="a", name=f"psv{sc}")
                    for e in range(NE):
                        nc.tensor.matmul(
                            ps_v[:], xs(j, e, sc * P, (sc + 1) * P),
                            wv_all[:, e * D:(e + 1) * D],
                            start=(e == 0), stop=(e == NE - 1))
                    nc.scalar.copy(v_sb[scg][:], ps_v[:])
                return qT

            def attention(j, qT):
                """one flat software-pipelined stream over (head, key-chunk)
                so the PE never drains at head ends."""
                nt = 4 * j + 4
                ot_blk = []
                outps, rbs_ps, sps = {}, {}, {}

                def offs(t):
                    return 0 if t < 4 * j else (t - 4 * j) * P

                def stage1(h, t):
                    o = offs(t)
                    sp = psA.tile([P, BS], f32, tag="a", name=f"sp{h}_{t}")
                    nc.tensor.matmul(sp[:, o:], kT_sb[:, t * P:(t + 1) * P],
                                     qT[h][:, o:], start=True, stop=True)
                    sps[(h, t)] = sp

                def stage2(h, t):
                    o = offs(t)
                    sp = sps.pop((h, t))
                    if t == 0:
                        outps[h] = psO.tile([P, BS], f32, tag="o",
                                            name=f"o{h}")
                        rbs_ps[h] = psY.tile([P, BS], f32, tag="y",
                                             name=f"r{h}")
                    ex = exp_p.tile([P, BS], bf16, tag="ex", name=f"ex{t}")
                    nc.scalar.activation(ex[:, o:], sp[:, o:], EXP, scale=SCALE)
                    if t >= 4 * j:
                        nc.gpsimd.tensor_mul(ex[:, o:o + P], ex[:, o:o + P],
                                             tri_sb[:])
                    nc.tensor.matmul(outps[h][:, o:], v_sb[t][:], ex[:, o:],
                                     start=(t == 0), stop=(t == nt - 1),
                                     skip_group_check=True)
                    # rowsum broadcast to all 128 partitions in one matmul
                    nc.tensor.matmul(rbs_ps[h][:, o:], ones_sb[:], ex[:, o:],
                                     start=(t == 0), stop=(t == nt - 1),
                                     skip_group_check=True)
                    if t == nt - 1:
                        rbs = rcp.tile([P, BS], f32, tag="rbs")
                        nc.vector.reciprocal_approx_fast(rbs[:],
                                                         rbs_ps.pop(h)[:])
                        ot = otp.tile([P, BS], bf16, tag="oT")
                        nc.vector.tensor_mul(ot[:], outps.pop(h)[:], rbs[:])
                        ot_blk.append(ot)

                seq = [(h, t) for h in range(G) for t in range(nt)]
                stage1(*seq[0])
                for i in range(1, len(seq)):
                    stage1(*seq[i])
                    stage2(*seq[i - 1])
                stage2(*seq[-1])
                return ot_blk

            def out_proj(j, ot_blk):
                yb = yp.tile([P, 4 * E], bf16, tag="y")
                for sc in range(4):
                    for eb in range(4):
                        ypn = psY.tile([P, BS], f32, tag="y", name=f"yp{eb}")
                        for h in range(G):
                            nc.tensor.matmul(
                                ypn[:],
                                ot_blk[h][:, sc * P:(sc + 1) * P],
                                wo_all[:, h * E + eb * BS:h * E + (eb + 1) * BS],
                                start=(h == 0), stop=(h == G - 1))
                        dst = yb[:, sc * E + eb * BS:sc * E + (eb + 1) * BS]
                        if eb % 2 == 0:
                            nc.scalar.copy(dst, ypn[:])
                        else:
                            nc.vector.tensor_copy(dst, ypn[:])
                            r0 = j * BS + sc * P
                            c0 = (eb - 1) * BS
                            nc.sync.dma_start(
                                y_d[r0:r0 + P, c0:c0 + 2 * BS],
                                yb[:, sc * E + c0:sc * E + c0 + 2 * BS])

            # main flow: next block's k-chain slots between attention and
            # out-projection so the PE has work across both boundaries
            qT = proj_qv(0, proj_k(0))
            for j in range(NB):
                ot_blk = attention(j, qT)
                if j + 1 < NB:
                    kctx = proj_k(j + 1)
                out_proj(j, ot_blk)
                if j + 1 < NB:
                    qT = proj_qv(j + 1, kctx)

    nc.compile()
    _CACHE["nc"] = nc
    return nc


def _tables():
    inv = 1.0 / THETA ** (np.arange(0, D, 2, dtype=np.float64) / D)   # [64]
    t = np.arange(S, dtype=np.float64)
    fr = np.outer(inv, t)                    # [64, S]
    csT = np.empty((P, S), dtype=np.float32)
    csT[0:64] = np.cos(fr)
    csT[64:128] = np.cos(fr)
    snT = np.empty((P, S), dtype=np.float32)
    snT[0:64] = np.sin(fr)
    snT[64:128] = np.sin(fr)
    # perm (as matmul lhsT): out[m] = -in[m+64] (m<64) ; +in[m-64] (m>=64)
    pm = np.zeros((P, P), dtype=np.float32)
    for m in range(64):
        pm[m + 64, m] = -1.0
        pm[m, m + 64] = 1.0
    # tri[p, c] = 1 if p <= c (within-chunk causal mask)
    tri = (np.arange(P)[:, None] <= np.arange(P)[None, :]).astype(np.float32)
    return (csT.astype(BF16), snT.astype(BF16),
            pm.astype(BF16), tri.astype(BF16))


def _pack_e(w):
    """[E, d] -> [P, NE*d]: chip layout, contiguous per partition."""
    d = w.shape[1]
    return np.ascontiguousarray(
        w.reshape(NE, P, d).transpose(1, 0, 2).reshape(P, NE * d))


def _in_maps(x, Wq, Wk, Wv, Wo):
    csT, snT, pm, tri = _tables()
    # xP[p, (j, e, s)] = x[b][j*BS+s, e*P+p]
    xP = [np.ascontiguousarray(
        x[b].T.reshape(NE, P, NB, BS).transpose(1, 2, 0, 3)
        .reshape(P, NB * NE * BS)).astype(BF16) for b in range(B)]
    maps = []
    for c in range(8):
        b, kv = c // 4, c % 4
        wo_s = Wo[kv * G * D:(kv + 1) * G * D, :]
        maps.append({
            "xP": xP[b],
            "wqP": _pack_e(Wq[:, kv * G * D:(kv + 1) * G * D]).astype(BF16),
            "wkP": _pack_e(Wk[:, kv * D:(kv + 1) * D]).astype(BF16),
            "wvP": _pack_e(Wv[:, kv * D:(kv + 1) * D]).astype(BF16),
            "woP": np.ascontiguousarray(
                wo_s.reshape(G, P, E).transpose(1, 0, 2)
                .reshape(P, G * E)).astype(BF16),
            "csT": csT, "snT": snT, "pm": pm, "tri": tri,
        })
    return maps


def _gather(results):
    out = np.empty((B, S, E), dtype=np.float32)
    for b in range(B):
        acc = results[4 * b]["y"].astype(np.float32)
        for kv in range(1, 4):
            acc += results[4 * b + kv]["y"].astype(np.float32)
        out[b] = acc
    return out


def run(x, Wq, Wk, Wv, Wo, trace=False, **trace_kwargs):
    from concourse.bass_utils import run_bass_kernel_spmd
    nc = _build()
    res = run_bass_kernel_spmd(nc, _in_maps(x, Wq, Wk, Wv, Wo),
                               list(range(8)), trace=trace, **trace_kwargs)
    return _gather(res.results), res


def kernel(x, Wq, Wk, Wv, Wo):
    out, _ = run(np.asarray(x), np.asarray(Wq), np.asarray(Wk),
                 np.asarray(Wv), np.asarray(Wo))
    return out


# revision 31
# speedup vs baseline: 1.0100x; 1.0100x over previous
"""GQA attention (B=2,S=2048,E=2048,H=16,KV=4,D=128, RoPE, causal) on 8 trn2 cores.

Sharding: core c = (b = c//4, kv = c%4). Tensor-parallel over kv-head groups
(Wq cols / Wk,Wv cols / Wo rows) x data-parallel over batch. Each core computes
a full [S, E] partial output (its head group's contribution); host sums the 4
partials per batch element.

All device inputs are bf16, host-packed to the on-chip layout ([128, ...]
with fully-contiguous partition rows) so every DMA is a fast same-dtype
hardware-DGE transfer; x on the sync queue, weights on the scalar queue.
The next block's k-projection chain is emitted between attention and the
output projection so the PE stays busy across phase boundaries. Every
matmul has its contraction dim on partitions:
  qT/kT [d, s] = Wq_chunk.T @ xT    (PSUM accum over e-chunks)
  RoPE rotation on-chip: rot = perm.T @ qraw (one [128,128] perm matmul)
  v     [s, d] = xT_chunk.T @ Wv
  scoresT [sk, sq] = kT_chunk.T @ qT_block  (suffix-width in diagonal strip)
  softmax without max-subtraction; rowsum+broadcast per key chunk in one
  PE matmul (ones[128,128] stationary, PSUM-accumulated over chunks);
  1/rowsum via reciprocal_approx_fast on [128,512]
  outT [d, sq] += v_chunk.T @ expT
  y [sq, e] += outT_norm_chunk.T @ Wo_head  (accum over 4 heads)

All matmuls bf16 (fp32 PSUM accum); y written bf16, host sums in fp32.
"""
import sys
sys.path.insert(0, "/opt/trn_rl_repo")
import numpy as np
import ml_dtypes

BF16 = ml_dtypes.bfloat16

B, S, E = 2, 2048, 2048
H, KV, D = 16, 4, 128
G = H // KV          # 4 q heads per kv head / core
THETA = 10000.0
P = 128
NE = E // P          # 16 e-chunks
NB = 4               # s-blocks per core loop
BS = S // NB         # 512
NSC = S // P         # 16 s-chunks

_CACHE = {}


def _build():
    if "nc" in _CACHE:
        return _CACHE["nc"]
    import concourse.bass as bass
    import concourse.tile as tile
    from concourse import mybir, bacc

    f32 = mybir.dt.float32
    bf16 = mybir.dt.bfloat16
    EXP = mybir.ActivationFunctionType.Exp
    SCALE = 1.0 / np.sqrt(D)

    nc = bacc.Bacc("TRN2", target_bir_lowering=False, debug=False)
    # all packed host-side to the on-chip layout: fully-contiguous rows
    xT_d = nc.declare_dram_parameter("xP", [P, NB * NE * BS], bf16,
                                     isOutput=False)
    wq_d = nc.declare_dram_parameter("wqP", [P, NE * G * D], bf16,
                                     isOutput=False)
    wk_d = nc.declare_dram_parameter("wkP", [P, NE * D], bf16, isOutput=False)
    wv_d = nc.declare_dram_parameter("wvP", [P, NE * D], bf16, isOutput=False)
    wo_d = nc.declare_dram_parameter("woP", [P, G * E], bf16, isOutput=False)
    cs_d = nc.declare_dram_parameter("csT", [P, S], bf16, isOutput=False)
    sn_d = nc.declare_dram_parameter("snT", [P, S], bf16, isOutput=False)
    pm_d = nc.declare_dram_parameter("pm", [P, P], bf16, isOutput=False)
    tri_d = nc.declare_dram_parameter("tri", [P, P], bf16, isOutput=False)
    y_d = nc.declare_dram_parameter("y", [S, E], bf16, isOutput=True)

    with tile.TileContext(nc) as tc, \
         nc.allow_low_precision(reason="bf16 matmul pipeline"):
        import contextlib
        with contextlib.ExitStack() as ctx:
            cst = ctx.enter_context(tc.tile_pool(name="cst", bufs=1))
            xp = ctx.enter_context(tc.tile_pool(name="xp", bufs=4))
            wp = ctx.enter_context(tc.tile_pool(name="wp", bufs=1))
            kvp = ctx.enter_context(tc.tile_pool(name="kvp", bufs=1))
            vp = ctx.enter_context(tc.tile_pool(name="vp", bufs=16))
            qtp = ctx.enter_context(tc.tile_pool(name="qtp", bufs=5))
            rawp = ctx.enter_context(tc.tile_pool(name="rawp", bufs=3))
            rpp = ctx.enter_context(tc.tile_pool(name="rpp", bufs=4))
            exp_p = ctx.enter_context(tc.tile_pool(name="exp", bufs=4))
            rcp = ctx.enter_context(tc.tile_pool(name="rcp", bufs=2))
            otp = ctx.enter_context(tc.tile_pool(name="otp", bufs=6))
            yp = ctx.enter_context(tc.tile_pool(name="yp", bufs=2))
            psA = ctx.enter_context(tc.tile_pool(name="psA", bufs=3, space="PSUM"))
            psO = ctx.enter_context(tc.tile_pool(name="psO", bufs=2, space="PSUM"))
            psY = ctx.enter_context(tc.tile_pool(name="psY", bufs=3, space="PSUM"))

            # ---- resident tiles ----
            xtj = [xp.tile([P, NE * BS], bf16, tag="xt", name=f"xt{j}")
                   for j in range(NB)]
            wq_all = wp.tile([P, NE * G * D], bf16, tag="wq")
            wk_all = wp.tile([P, NE * D], bf16, tag="wk")
            wv_all = wp.tile([P, NE * D], bf16, tag="wv")
            wo_all = wp.tile([P, G * E], bf16, tag="wo")
            cs_sb = cst.tile([P, S], bf16, tag="cs")
            sn_sb = cst.tile([P, S], bf16, tag="sn")
            pm_sb = cst.tile([P, P], bf16, tag="pm")
            tri_sb = cst.tile([P, P], bf16, tag="tri")
            ones_sb = cst.tile([P, P], bf16, tag="ones")
            nc.vector.memset(ones_sb[:], 1.0)
            kT_sb = kvp.tile([P, S], bf16, tag="kT")
            v_sb = [vp.tile([P, D], bf16, tag="v", name=f"v{i}")
                    for i in range(NSC)]

            def xs(j, e, lo=0, hi=BS):
                """moving slice of block-j x panel, e-chunk rows."""
                return xtj[j][:, e * BS + lo:e * BS + hi]

            # ---- DMA: everything is a fully-contiguous 2D transfer
            # (arrays are host-packed to the on-chip layout). x on the
            # sync queue, weights on the scalar queue. ----
            def load_x(j, e0, e1):
                lo, hi = j * NE * BS + e0 * BS, j * NE * BS + e1 * BS
                nc.sync.dma_start(xtj[j][:, e0 * BS:e1 * BS], xT_d[:, lo:hi])

            GD = G * D
            load_x(0, 0, 4)
            nc.sync.dma_start(wk_all[:, 0:4 * D], wk_d[:, 0:4 * D])
            load_x(0, 4, 8)
            nc.sync.dma_start(wk_all[:, 4 * D:], wk_d[:, 4 * D:])
            load_x(0, 8, 12)
            nc.scalar.dma_start(pm_sb[:], pm_d[:])
            nc.scalar.dma_start(tri_sb[:], tri_d[:])
            nc.scalar.dma_start(wq_all[:, 0:4 * GD], wq_d[:, 0:4 * GD])
            nc.scalar.dma_start(xtj[0][:, 12 * BS:16 * BS],
                                xT_d[:, 12 * BS:16 * BS])
            nc.sync.dma_start(cs_sb[:], cs_d[:])
            nc.sync.dma_start(sn_sb[:], sn_d[:])
            for q in range(1, 4):
                nc.scalar.dma_start(wq_all[:, 4 * q * GD:4 * (q + 1) * GD],
                                    wq_d[:, 4 * q * GD:4 * (q + 1) * GD])
            for j in range(1, NB):
                load_x(j, 0, NE)
            nc.scalar.dma_start(wv_all[:], wv_d[:])
            nc.scalar.dma_start(wo_all[:], wo_d[:])

            def rope_evac(dst, ps, ps2, j):
                """dst (bf16) = ps*cos + ps2*sin at abs position j*BS."""
                cs = cs_sb[:, j * BS:(j + 1) * BS]
                sn = sn_sb[:, j * BS:(j + 1) * BS]
                t1 = rpp.tile([P, BS], f32, tag="rp")
                nc.vector.tensor_mul(t1[:], ps[:], cs)
                t2 = rpp.tile([P, BS], f32, tag="rp")
                nc.vector.tensor_mul(t2[:], ps2[:], sn)
                nc.vector.tensor_add(dst, t1[:], t2[:])

            def proj_k(j):
                js = slice(j * BS, (j + 1) * BS)
                ps_k = psA.tile([P, BS], f32, tag="a", name=f"psk{j}")
                for e in range(NE):
                    nc.tensor.matmul(ps_k[:], wk_all[:, e * D:(e + 1) * D],
                                     xs(j, e),
                                     start=(e == 0), stop=(e == NE - 1))
                kraw = rawp.tile([P, BS], bf16, tag="raw", name=f"kraw{j}")
                nc.scalar.copy(kraw[:], ps_k[:])
                return ps_k, kraw

            def proj_qv(j, kctx):
                """q chains with trailing perm+rope (starting with k's),
                then v chains."""
                js = slice(j * BS, (j + 1) * BS)
                ps_k, kraw = kctx
                qT = []
                prev = ("k", ps_k, kraw, None)
                for h in range(G):
                    ps_q = psA.tile([P, BS], f32, tag="a", name=f"psq{h}")
                    for e in range(NE):
                        nc.tensor.matmul(
                            ps_q[:],
                            wq_all[:, e * G * D + h * D:e * G * D + (h + 1) * D],
                            xs(j, e),
                            start=(e == 0), stop=(e == NE - 1))
                    qraw = rawp.tile([P, BS], bf16, tag="raw", name=f"qraw{h}")
                    nc.scalar.copy(qraw[:], ps_q[:])
                    kind, psp, raw, dst = prev
                    ps2 = psA.tile([P, BS], f32, tag="a", name=f"ps2{h}")
                    nc.tensor.matmul(ps2[:], pm_sb[:], raw[:],
                                     start=True, stop=True)
                    if kind == "k":
                        rope_evac(kT_sb[:, js], psp, ps2, j)
                    else:
                        rope_evac(dst[:], psp, ps2, j)
                    qh = qtp.tile([P, BS], bf16, tag="qT", name=f"qT{h}")
                    qT.append(qh)
                    prev = ("q", ps_q, qraw, qh)

                kind, psp, raw, dst = prev
                ps2 = psA.tile([P, BS], f32, tag="a", name="ps2l")
                nc.tensor.matmul(ps2[:], pm_sb[:], raw[:], start=True,
                                 stop=True)
                rope_evac(dst[:], psp, ps2, j)

                for sc in range(4):
                    scg = 4 * j + sc
                    ps_v = psA.tile([P, D], f32, tag="a", name=f"psv{sc}")
                    for e in range(NE):
                        nc.tensor.matmul(
                            ps_v[:], xs(j, e, sc * P, (sc + 1) * P),
                            wv_all[:, e * D:(e + 1) * D],
                            start=(e == 0), stop=(e == NE - 1))
                    nc.scalar.copy(v_sb[scg][:], ps_v[:])
                return qT

            def attention(j, qT):
                """one flat software-pipelined stream over (head, key-chunk)
                so the PE never drains at head ends."""
                nt = 4 * j + 4
                ot_blk = []
                outps, rbs_ps, sps = {}, {}, {}

                def offs(t):
                    return 0 if t < 4 * j else (t - 4 * j) * P

                def stage1(h, t):
                    o = offs(t)
                    sp = psA.tile([P, BS], f32, tag="a", name=f"sp{h}_{t}")
                    nc.tensor.matmul(sp[:, o:], kT_sb[:, t * P:(t + 1) * P],
                                     qT[h][:, o:], start=True, stop=True)
                    sps[(h, t)] = sp

                def stage2(h, t):
                    o = offs(t)
                    sp = sps.pop((h, t))
                    if t == 0:
                        outps[h] = psO.tile([P, BS], f32, tag="o",
                                            name=f"o{h}")
                        rbs_ps[h] = psY.tile([P, BS], f32, tag="y",
                                             name=f"r{h}")
                    ex = exp_p.tile([P, BS], bf16, tag="ex", name=f"ex{t}")
                    nc.scalar.activation(ex[:, o:], sp[:, o:], EXP, scale=SCALE)
                    if t >= 4 * j:
                        nc.gpsimd.tensor_mul(ex[:, o:o + P], ex[:, o:o + P],
                                             tri_sb[:])
                    nc.tensor.matmul(outps[h][:, o:], v_sb[t][:], ex[:, o:],
                                     start=(t == 0), stop=(t == nt - 1),
                                     skip_group_check=True)
                    # rowsum broadcast to all 128 partitions in one matmul
                    nc.tensor.matmul(rbs_ps[h][:, o:], ones_sb[:], ex[:, o:],
                                     start=(t == 0), stop=(t == nt - 1),
                                     skip_group_check=True)
                    if t == nt - 1:
                        rbs = rcp.tile([P, BS], f32, tag="rbs")
                        nc.vector.reciprocal_approx_fast(rbs[:],
                                                         rbs_ps.pop(h)[:])
                        ot = otp.tile([P, BS], bf16, tag="oT")
                        nc.vector.tensor_mul(ot[:], outps.pop(h)[:], rbs[:])
                        ot_blk.append(ot)

                seq = [(h, t) for h in range(G) for t in range(nt)]
                stage1(*seq[0])
                for i in range(1, len(seq)):
                    stage1(*seq[i])
                    stage2(*seq[i - 1])
                stage2(*seq[-1])
                return ot_blk

            def out_proj(j, ot_blk):
                yb = yp.tile([P, 4 * E], bf16, tag="y")
                for sc in range(4):
                    for eb in range(4):
                        ypn = psY.tile([P, BS], f32, tag="y", name=f"yp{eb}")
                        for h in range(G):
                            nc.tensor.matmul(
                                ypn[:],
                                ot_blk[h][:, sc * P:(sc + 1) * P],
                                wo_all[:, h * E + eb * BS:h * E + (eb + 1) * BS],
                                start=(h == 0), stop=(h == G - 1))
                        dst = yb[:, sc * E + eb * BS:sc * E + (eb + 1) * BS]
                        if eb % 2 == 0:
                            nc.scalar.copy(dst, ypn[:])
                        else:
                            nc.vector.tensor_copy(dst, ypn[:])
                            r0 = j * BS + sc * P
                            c0 = (eb - 1) * BS
                            nc.sync.dma_start(
                                y_d[r0:r0 + P, c0:c0 + 2 * BS],
                                yb[:, sc * E + c0:sc * E + c0 + 2 * BS])

            # main flow: next block's k-chain slots between attention and
            # out-projection so the PE has work across both boundaries
            qT = proj_qv(0, proj_k(0))
            for j in range(NB):
                ot_blk = attention(j, qT)
                if j + 1 < NB:
                    kctx = proj_k(j + 1)
                out_proj(j, ot_blk)
                if j + 1 < NB:
                    qT = proj_qv(j + 1, kctx)

    nc.compile()
    _CACHE["nc"] = nc
    return nc


def _tables():
    inv = 1.0 / THETA ** (np.arange(0, D, 2, dtype=np.float64) / D)   # [64]
    t = np.arange(S, dtype=np.float64)
    fr = np.outer(inv, t)                    # [64, S]
    csT = np.empty((P, S), dtype=np.float32)
    csT[0:64] = np.cos(fr)
    csT[64:128] = np.cos(fr)
    snT = np.empty((P, S), dtype=np.float32)
    snT[0:64] = np.sin(fr)
    snT[64:128] = np.sin(fr)
    # perm (as matmul lhsT): out[m] = -in[m+64] (m<64) ; +in[m-64] (m>=64)
    pm = np.zeros((P, P), dtype=np.float32)
    for m in range(64):
        pm[m + 64, m] = -1.0
        pm[m, m + 64] = 1.0
    # tri[p, c] = 1 if p <= c (within-chunk causal mask)
    tri = (np.arange(P)[:, None] <= np.arange(P)[None, :]).astype(np.float32)
    return (csT.astype(BF16), snT.astype(BF16),
            pm.astype(BF16), tri.astype(BF16))


def _pack_e(w):
    """[E, d] -> [P, NE*d]: chip layout, contiguous per partition."""
    d = w.shape[1]
    return np.ascontiguousarray(
        w.reshape(NE, P, d).transpose(1, 0, 2).reshape(P, NE * d))


def _in_maps(x, Wq, Wk, Wv, Wo):
    csT, snT, pm, tri = _tables()
    # xP[p, (j, e, s)] = x[b][j*BS+s, e*P+p]
    xP = [np.ascontiguousarray(
        x[b].T.reshape(NE, P, NB, BS).transpose(1, 2, 0, 3)
        .reshape(P, NB * NE * BS)).astype(BF16) for b in range(B)]
    maps = []
    for c in range(8):
        b, kv = c // 4, c % 4
        wo_s = Wo[kv * G * D:(kv + 1) * G * D, :]
        maps.append({
            "xP": xP[b],
            "wqP": _pack_e(Wq[:, kv * G * D:(kv + 1) * G * D]).astype(BF16),
            "wkP": _pack_e(Wk[:, kv * D:(kv + 1) * D]).astype(BF16),
            "wvP": _pack_e(Wv[:, kv * D:(kv + 1) * D]).astype(BF16),
            "woP": np.ascontiguousarray(
                wo_s.reshape(G, P, E).transpose(1, 0, 2)
                .reshape(P, G * E)).astype(BF16),
            "csT": csT, "snT": snT, "pm": pm, "tri": tri,
        })
    return maps


def _gather(results):
    out = np.empty((B, S, E), dtype=np.float32)
    for b in range(B):
        acc = results[4 * b]["y"].astype(np.float32)
        for kv in range(1, 4):
            acc += results[4 * b + kv]["y"].astype(np.float32)
        out[b] = acc
    return out


def run(x, Wq, Wk, Wv, Wo, trace=False, **trace_kwargs):
    from concourse.bass_utils import run_bass_kernel_spmd
    nc = _build()
    res = run_bass_kernel_spmd(nc, _in_maps(x, Wq, Wk, Wv, Wo),
                               list(range(8)), trace=trace, **trace_kwargs)
    return _gather(res.results), res


def kernel(x, Wq, Wk, Wv, Wo):
    out, _ = run(np.asarray(x), np.asarray(Wq), np.asarray(Wk),
                 np.asarray(Wv), np.asarray(Wo))
    return out
